# revision 46
# baseline (speedup 1.0000x reference)
"""DenseGCNLayer (GCNConv + BatchNorm + ReLU) on 8 TRN2 NeuronCores.

Self-contained kernel: takes the FULL inputs, shards target nodes across 8
cores, runs a raw-bass SPMD program (bf16 compute, f32 accumulation), returns
the full [N, D] float32 output.

Strategy (v2, no h-AllGather): the linear transform commutes with the
aggregation, so compute  s[c] = dinv[c] * (sum_{r->c} dinv[r] x[r])  first
(self-loops folded into the edge list), then  agg = s @ W.T,  then BatchNorm
(global stats via a tiny [P,4] AllReduce) + ReLU.  Every core gets the SAME
replicated gather table  xs = dinv[:,None]*x  (bf16, host-prepared), so the
only on-device communication is the 2KB BN-stats AllReduce.

Per core: 6250 target nodes in 49 tiles of 128; incoming edges (+self loops)
are grouped per (tile, src-half), padded to 128-edge blocks, streamed via
1024-row dma_gather chunks (4 SWDGE queues) into two 48-block ring buffers.
Each 128-edge block is reduced into its target tile by a PE matmul with a
0/1 one-hot (edge-slot -> target-slot) built on DVE.  Per tile, PE also
transposes s (via identity matmul) and applies W with stationary W chunks;
ACT squares agg for variance stats and runs the fused relu(A*x+B) epilogue
into one big bf16 buffer that is stored in 8-tile DMA batches.  Output is
produced transposed ([2,128,S] bf16 per core) and fixed up on host.
"""
from contextlib import ExitStack

import numpy as np
import ml_dtypes

import concourse.bass as bass
import concourse.bacc as bacc
import concourse.mybir as mybir
from concourse.library_config import mlp

P = 128
GCH = 8        # blocks per gather instruction
RB = 48         # gather ring capacity per stream, in 128-edge blocks
GW = 8          # one-hot blocks built per DVE op
NSEM = 8        # rotating DMA sems per stream
OHR = 8         # one-hot ring depth (groups)
NQ = 4          # SWDGE queues
SR = 4          # s ring
SR2 = 4         # sT ring
BN_EPS = 1e-5
BF16 = ml_dtypes.bfloat16
NCONST = 7      # small const loads (Wt x2, gbc, iota, ident, ones1, dinv)
CHEAD = 8       # idx-map head chunks per stream loaded before gathers start
DEBUG = False   # add a dbg output dumping stats/coef buffers + agg tiles


# ---------------------------------------------------------------- host prep

def _preprocess(x, edge_index, W, gamma, beta, M=8):
    N, D = x.shape
    S = N // M
    assert S * M == N
    T = (S + P - 1) // P
    NH = N // 2
    src_e = np.asarray(edge_index[0], np.int64)
    tgt_e = np.asarray(edge_index[1], np.int64)
    deg = (np.bincount(tgt_e, minlength=N) + 1).astype(np.float64)
    dinv = (1.0 / np.sqrt(deg)).astype(np.float32)

    # fold self-loops into the edge stream
    loops = np.arange(N, dtype=np.int64)
    src = np.concatenate([src_e, loops])
    tgt = np.concatenate([tgt_e, loops])

    core_of = tgt // S
    loc = tgt - core_of * S
    tl = loc // P
    slot = loc % P
    ishi = (src >= NH).astype(np.int64)
    key = (core_of * T + tl) * 2 + ishi
    order = np.argsort(key, kind="stable")
    cnt = np.bincount(key, minlength=M * T * 2).reshape(M, T, 2)
    starts = np.zeros(M * T * 2 + 1, np.int64)
    np.cumsum(cnt.reshape(-1), out=starts[1:])

    nblk = -(-cnt // P)                       # ceil, [M, T, 2]
    Blo = np.maximum(nblk[:, :, 0].max(axis=0), 1)   # [T]
    Bhi = np.maximum(nblk[:, :, 1].max(axis=0), 1)
    assert Blo.max() <= RB and Bhi.max() <= RB
    LB, HB = int(Blo.sum()), int(Bhi.sum())
    lo_start = np.zeros(T, np.int64); np.cumsum(Blo[:-1], out=lo_start[1:])
    hi_start = np.zeros(T, np.int64); np.cumsum(Bhi[:-1], out=hi_start[1:])

    # consumption order: per tile, lo blocks then hi blocks
    cons = []
    for t in range(T):
        for i in range(int(Blo[t])):
            cons.append((0, int(lo_start[t] + i), t))
        for i in range(int(Bhi[t])):
            cons.append((1, int(hi_start[t] + i), t))
    NBLK = len(cons)

    # gather schedule: fixed GCH-block chunk instructions per stream (ring
    # position == stream block index, no skips), issued in consumption
    # order of the first block of each chunk.
    LBc = -(-LB // GCH)
    HBc = -(-HB // GCH)
    runs = []                      # (st, chunk_id, b0, nb, rpos)
    first_need = {}
    for step, (st, sp, _t) in enumerate(cons):
        key = (st, sp // GCH)
        if key not in first_need:
            first_need[key] = step
    for st, ch in sorted(first_need, key=first_need.get):
        nblks = (LBc, HBc)[st] * 0 + min(GCH, (LB, HB)[st] - ch * GCH)
        runs.append((st, ch, ch * GCH, int(nblks), ch * GCH))
    assert len(runs) == LBc + HBc

    shared = {}
    xs = (np.asarray(x, np.float32) * dinv[:, None]).astype(BF16)
    shared["tab"] = np.ascontiguousarray(xs.reshape(2, NH, D))
    shared["Wt"] = np.ascontiguousarray(W.T.astype(BF16)).reshape(2, P, D)
    gbc = np.zeros((P, 4), np.float32)
    gbc[:, 0], gbc[:, 1] = gamma[:P], gamma[P:]
    gbc[:, 2], gbc[:, 3] = beta[:P], beta[P:]
    shared["gbc"] = gbc
    shared["iota"] = np.ascontiguousarray(
        np.broadcast_to(np.tile(np.arange(P), GW).astype(BF16), (P, GW * P))
    )
    shared["ident"] = np.eye(P, dtype=BF16)
    shared["ones1"] = np.ones((P, 1), BF16)

    in_maps = []
    for m in range(M):
        # pad gather slots with row 0 (seg=-1 keeps them out of the one-hot)
        lo_src = np.empty(LB * P, np.int16)
        lo_slot = np.full(LB * P, -1.0, np.float32)
        hi_src = np.empty(HB * P, np.int16)
        hi_slot = np.full(HB * P, -1.0, np.float32)
        lo_src[:] = 0
        hi_src[:] = 0
        for t in range(T):
            for h, bsrc, bslot, bstart in (
                (0, lo_src, lo_slot, lo_start[t]),
                (1, hi_src, hi_slot, hi_start[t]),
            ):
                k = (m * T + t) * 2 + h
                e = order[starts[k] : starts[k + 1]]
                n = len(e)
                off = int(bstart) * P
                bsrc[off : off + n] = (src[e] - (NH if h else 0)).astype(np.int16)
                bslot[off : off + n] = slot[e].astype(np.float32)

        def wrap_idx(flat):
            # idx j -> partition j%16, col j//16; replicate x8 across partitions
            a = flat.reshape(-1, 16).T                               # [16, cols]
            out = np.concatenate([a] * 8, axis=0)                    # [128, cols]
            return np.ascontiguousarray(out)

        seg = np.full((P, NBLK), -1.0, np.float32)
        for g_blk, (st, sp, _t) in enumerate(cons):
            arr = lo_slot if st == 0 else hi_slot
            seg[:, g_blk] = arr[sp * P : (sp + 1) * P]

        dinv_t = np.ones((P, T), np.float32)
        dm = dinv[m * S : (m + 1) * S]
        for t in range(T):
            rows = min(P, S - t * P)
            dinv_t[:rows, t] = dm[t * P : t * P + rows]

        in_maps.append(
            dict(
                idx_lo=wrap_idx(lo_src), idx_hi=wrap_idx(hi_src),
                seg=seg.astype(BF16), dinv_t=dinv_t, **shared,
            )
        )

    meta = dict(
        N=N, D=D, M=M, S=S, T=T, NH=NH,
        Blo=Blo, Bhi=Bhi, LB=LB, HB=HB,
        lo_start=lo_start, hi_start=hi_start, cons=cons,
        runs=runs,
    )
    return in_maps, meta


# ------------------------------------------------------------- bass program

def _build_program(meta, REP=1):
    N, D, S, T = meta["N"], meta["D"], meta["S"], meta["T"]
    NH, M = meta["NH"], meta["M"]
    cons, runs = meta["cons"], meta["runs"]
    Blo, Bhi = meta["Blo"], meta["Bhi"]
    lo_start, hi_start = meta["lo_start"], meta["hi_start"]
    LB, HB = meta["LB"], meta["HB"]
    NBLK = len(cons)
    NGRP = -(-NBLK // GW)
    bf = mybir.dt.bfloat16
    f32 = mybir.dt.float32

    nc = bacc.Bacc(num_devices=M, num_swdge_queues=NQ, detect_race_conditions=False)

    tab_d = nc.declare_dram_parameter("tab", [2, NH, D], bf, isOutput=False)
    idx_lo_d = nc.declare_dram_parameter("idx_lo", [P, LB * 8], mybir.dt.int16, isOutput=False)
    idx_hi_d = nc.declare_dram_parameter("idx_hi", [P, HB * 8], mybir.dt.int16, isOutput=False)
    seg_d = nc.declare_dram_parameter("seg", [P, NBLK], bf, isOutput=False)
    dinv_d = nc.declare_dram_parameter("dinv_t", [P, T], f32, isOutput=False)
    Wt_d = nc.declare_dram_parameter("Wt", [2, P, D], bf, isOutput=False)
    gbc_d = nc.declare_dram_parameter("gbc", [P, 4], f32, isOutput=False)
    iota_d = nc.declare_dram_parameter("iota", [P, GW * P], bf, isOutput=False)
    ident_d = nc.declare_dram_parameter("ident", [P, P], bf, isOutput=False)
    ones1_d = nc.declare_dram_parameter("ones1", [P, 1], bf, isOutput=False)
    y_d = nc.declare_dram_parameter("y", [2, P, S], bf, isOutput=True)
    if DEBUG:
        dbg_d = nc.declare_dram_parameter("dbg", [P, 20 + 3 * D], f32, isOutput=True)

    st_in = nc.dram_tensor("st_in", [P, 4], f32)
    st_out = nc.dram_tensor("st_out", [P, 4], f32, addr_space="Shared")

    with ExitStack() as ctx:
        sb = lambda name, shape, dt: ctx.enter_context(nc.sbuf_tensor(name, shape, dt))
        ring_lo = sb("rlo", [P, RB * D], bf)
        ring_hi = sb("rhi", [P, RB * D], bf)
        oh_sb = [sb(f"oh{r}", [P, GW * P], bf) for r in range(OHR)]
        s_sb = [sb(f"s{r}", [P, D], bf) for r in range(SR)]
        sT_sb = [sb(f"sT{r}", [P, D], bf) for r in range(SR2)]
        agg_sb = sb("agg_sb", [P, T * D], f32)
        sqcols = sb("sqcols", [P, 2 * T], f32)
        sqscr = sb("sqscr", [P, P], f32)
        idx_lo_sb = sb("idx_lo_sb", [P, LB * 8], mybir.dt.int16)
        idx_hi_sb = sb("idx_hi_sb", [P, HB * 8], mybir.dt.int16)
        seg_sb = sb("seg_sb", [P, NBLK], bf)
        dinv_sb = sb("dinv_sb", [P, T], f32)
        Wt_sb = sb("Wt_sb", [P, 2 * D], bf)
        gbc_sb = sb("gbc_sb", [P, 4], f32)
        iota_sb = sb("iota_sb", [P, GW * P], bf)
        ident_sb = sb("ident_sb", [P, P], bf)
        ones1_sb = sb("ones1_sb", [P, 1], bf)
        t1_sb = sb("t1_sb", [1, D], bf)
        mu_s_sb = sb("mu_s_sb", [P, 2], bf)
        stin_sb = sb("stin_sb", [P, 4], f32)
        stout_sb = sb("stout_sb", [P, 4], f32)
        tmp1_sb = sb("tmp1_sb", [P, 2], f32)
        tmp2_sb = sb("tmp2_sb", [P, 2], f32)
        mu_a_sb = sb("mu_a_sb", [P, 2], f32)
        msf_sb = sb("msf_sb", [P, 2], f32)
        rstd_sb = sb("rstd_sb", [P, 2], f32)
        ab_sb = sb("ab_sb", [P, 4], f32)
        y_big = sb("y_big", [P, 2 * T * P], bf)  # col = k*T*P + t*P + n

        pagg = [ctx.enter_context(nc.psum_tensor(f"pagg{r}", [P, D], f32)) for r in range(2)]
        psT = [ctx.enter_context(nc.psum_tensor(f"psT{r}", [P, D], bf)) for r in range(2)]
        paggT = [ctx.enter_context(nc.psum_tensor(f"paggT{r}", [P, D], f32)) for r in range(2)]
        ps1 = ctx.enter_context(nc.psum_tensor("ps1", [1, D], f32))
        pcol = ctx.enter_context(nc.psum_tensor("pcol", [P, 4], f32))

        sem = lambda name: ctx.enter_context(nc.semaphore(name))
        s_ld = sem("s_ld")
        s_ld2 = sem("s_ld2")
        s_ld3 = sem("s_ld3")
        s_ld4 = sem("s_ld4")
        s_glo = [sem(f"s_glo{r}") for r in range(NSEM)]
        s_ghi = [sem(f"s_ghi{r}") for r in range(NSEM)]
        s_clo = sem("s_clo")
        s_chi = sem("s_chi")
        s_oh = sem("s_oh")
        s_ohc = sem("s_ohc")
        s_pa = sem("s_pa")      # PE: pagg[t] accumulated
        s_s = sem("s_s")        # DVE: s_sb[t] written (also frees pagg[t])
        s_t = sem("s_t")        # PE: psT[t] written (also frees s_sb[t])
        s_tc = sem("s_tc")      # DVE: sT_sb[t] written (also frees psT[t])
        s_w = sem("s_w")        # PE: paggT[t] written
        s_ac = sem("s_ac")      # DVE: agg_sb tile t written (frees paggT[t])
        s_sq = sem("s_sq")      # ACT: square chunk done
        s_st = sem("s_st")      # PE: ps1 stats accumulation stopped
        s_t1 = sem("s_t1")      # DVE: t1_sb (=ps1) copied
        s_pscol = sem("s_pscol")
        s_stsb = sem("s_stsb")
        s_stst = sem("s_stst")
        s_ldst = sem("s_ldst")
        s_mu = sem("s_mu")
        s_pmu = sem("s_pmu")
        s_var = sem("s_var")
        s_rstd = sem("s_rstd")
        s_ab = sem("s_ab")
        s_yr = sem("s_yr")
        s_ye = sem("s_ye")
        cc = sem("cc")
        b1 = ctx.enter_context(nc.semaphore("b1"))
        b2 = ctx.enter_context(nc.semaphore("b2"))
        work_sems = (
            [s_ld, s_ld2, s_ld3, s_ld4, s_clo, s_chi, s_oh, s_ohc, s_pa, s_s, s_t, s_tc,
             s_w, s_ac, s_sq, s_st, s_t1, s_pscol, s_stsb, s_stst, s_ldst,
             s_mu, s_pmu, s_var, s_rstd, s_ab, s_yr, s_ye, cc]
            + s_glo + s_ghi
        )

        def _barrier(eng, it, clear=False):
            if REP == 1:
                return
            eng.sem_inc(b1, 1)
            eng.wait_ge(b1, 5 * (it + 1))
            if clear:
                for ws in work_sems:
                    eng.sem_clear(ws)
                eng.sem_inc(b2, 1)
            eng.wait_ge(b2, it + 1)

        block = ctx.enter_context(nc.Block())

        ST = 8  # tiles per output store DMA
        NSTG = -(-T // ST)

        # ---------------- SP: all HWDGE loads/stores
        @block.sync
        def _(sync):
            for _it in range(REP):
                hc = CHEAD * GCH * 8   # idx head columns per stream
                for dram, sbuf in ((idx_lo_d, idx_lo_sb), (idx_hi_d, idx_hi_sb)):
                    w = min(hc, dram.shape[1])
                    sync.dma_start(out=sbuf[:, 0:w], in_=dram[:, 0:w]).then_inc(
                        s_ld3, 16
                    )
                sync.dma_start(out=seg_sb[:], in_=seg_d[:]).then_inc(s_ld4, 16)
                for k in range(2):
                    sync.dma_start(
                        out=Wt_sb[:, k * D : (k + 1) * D], in_=Wt_d[k]
                    ).then_inc(s_ld, 16)
                for dram, sbuf in (
                    (gbc_d, gbc_sb),
                    (iota_d, iota_sb),
                    (ident_d, ident_sb),
                    (ones1_d, ones1_sb),
                    (dinv_d, dinv_sb),
                ):
                    sync.dma_start(out=sbuf[:], in_=dram[:]).then_inc(s_ld, 16)
                for dram, sbuf in ((idx_lo_d, idx_lo_sb), (idx_hi_d, idx_hi_sb)):
                    if dram.shape[1] > hc:
                        sync.dma_start(
                            out=sbuf[:, hc:], in_=dram[:, hc:]
                        ).then_inc(s_ld2, 16)
                    else:
                        sync.sem_inc(s_ld2, 16)
                sync.wait_ge(s_stsb, 3)
                sync.dma_start(out=st_in[:], in_=stin_sb[:]).then_inc(s_stst, 16)
                sync.wait_ge(cc, 1)
                sync.dma_start(out=stout_sb[:], in_=st_out[:]).then_inc(s_ldst, 16)
                for g in range(NSTG):
                    t1g = min(T, (g + 1) * ST)
                    c0, c1 = g * ST * P, min(S, t1g * P)
                    sync.wait_ge(s_yr, g + 1)
                    sync.dma_start(
                        out=y_d[:, :, c0:c1].rearrange("k p m -> p k m"),
                        in_=y_big[:].rearrange("p (k m) -> p k m", k=2)[:, :, c0:c1],
                    ).then_inc(s_ye, 16)
                sync.wait_ge(s_ye, 16 * NSTG)
                if DEBUG:
                    sync.wait_ge(s_ab, 1)
                    for off, sbuf in (
                        (0, stin_sb), (4, stout_sb), (8, ab_sb), (12, rstd_sb),
                        (14, tmp1_sb), (16, mu_a_sb), (18, msf_sb),
                    ):
                        w = sbuf.shape[1]
                        sync.dma_start(
                            out=dbg_d[:, off : off + w], in_=sbuf[:]
                        ).then_inc(s_ye, 16)
                    sync.dma_start(
                        out=dbg_d[:, 20 : 20 + D], in_=agg_sb[:, 0:D]
                    ).then_inc(s_ye, 16)
                    sync.dma_start(
                        out=dbg_d[:, 20 + D : 20 + 2 * D],
                        in_=agg_sb[:, (T - 1) * D : T * D],
                    ).then_inc(s_ye, 16)
                    sync.dma_start(
                        out=dbg_d[:, 20 + 2 * D : 20 + 3 * D],
                        in_=agg_sb[:, D : 2 * D],
                    ).then_inc(s_ye, 16)
                    sync.wait_ge(s_ye, 16 * (NSTG + 10))
                _barrier(sync, _it, clear=False)

        # ---------------- Pool: gathers + stats collective
        @block.gpsimd
        def _(gpsimd):
            for _it in range(REP):
                gpsimd.load_library(mlp)
                gpsimd.wait_ge(s_ld3, 32)
                tail_waited = False
                for r, (st, ch, b0, nb, rpos) in enumerate(runs):
                    if ch >= CHEAD and not tail_waited:
                        gpsimd.wait_ge(s_ld2, 32)
                        tail_waited = True
                    if b0 + nb > RB:
                        gpsimd.wait_ge(s_clo if st == 0 else s_chi, b0 + nb - RB)
                    idx_sb = idx_lo_sb if st == 0 else idx_hi_sb
                    ring = ring_lo if st == 0 else ring_hi
                    col0 = (b0 % RB) * D
                    gpsimd.dma_gather(
                        ring[:, col0 : col0 + nb * D].rearrange("p (k d) -> p k d", d=D),
                        tab_d[st],
                        idx_sb[:, b0 * 8 : (b0 + nb) * 8],
                        nb * P,
                        nb * P,
                        D,
                        queue_num=r % NQ,
                    ).then_inc((s_glo if st == 0 else s_ghi)[ch % NSEM], 16)
                gpsimd.wait_ge(s_stst, 16)
                gpsimd.collective_compute(
                    "AllReduce",
                    mybir.AluOpType.add,
                    replica_groups=[list(range(M))],
                    ins=[st_in[:]],
                    outs=[st_out[:]],
                ).then_inc(cc, 1)
                _barrier(gpsimd, _it, clear=True)

        # ---------------- PE
        @block.tensor
        def _(tensor):
            for _it in range(REP):
                tensor.wait_ge(s_ld, 16 * NCONST)
                g_blk = 0
                waited_ch = [-1, -1]
                pending = []

                def attach(mm, *incs):
                    # matmul sync-update slots are limited to 1; overflow rides
                    # the next matmul (consumers only ever see a later inc).
                    todo = pending + list(incs)
                    pending.clear()
                    for semh, v in todo[:1]:
                        mm.then_inc(semh, v)
                    pending.extend(todo[1:])

                def flush():
                    for semh, v in pending:
                        tensor.drain().then_inc(semh, v)
                    pending.clear()

                def wapply(u):
                    tensor.wait_ge(s_tc, u + 1)
                    if u >= 2:
                        tensor.wait_ge(s_ac, u - 1)
                    for j in range(2):
                        for i in range(2):
                            mm = tensor.matmul(
                                paggT[u % 2][:, j * P : (j + 1) * P],
                                Wt_sb[:, i * D + j * P : i * D + j * P + P],
                                sT_sb[u % SR2][:, i * P : (i + 1) * P],
                                start=(i == 0), stop=(i == 1),
                            )
                            attach(mm, *([(s_w, 1)] if (j == 1 and i == 1) else []))

                def extras(u):
                    if u >= 1:
                        wapply(u - 1)
                    tensor.wait_ge(s_s, u + 1)
                    mm = tensor.matmul(
                        ps1[:], ones1_sb[:], s_sb[u % SR][:, 0:D],
                        start=(u == 0), stop=(u == T - 1),
                    )
                    attach(mm, *([(s_st, 1)] if u == T - 1 else []))
                    if u >= 2:
                        tensor.wait_ge(s_tc, u - 1)
                    for k in range(2):
                        mm = tensor.transpose(
                            psT[u % 2][:, k * P : (k + 1) * P],
                            s_sb[u % SR][:, k * P : (k + 1) * P],
                            ident_sb[:],
                        )
                        attach(mm, *([(s_t, 1)] if k == 1 else []))

                for t in range(T):
                    nbt = int(Blo[t] + Bhi[t])
                    done = 0
                    for st, base, num in ((0, lo_start[t], Blo[t]), (1, hi_start[t], Bhi[t])):
                        ring = ring_lo if st == 0 else ring_hi
                        NS = LB if st == 0 else HB
                        for i in range(int(num)):
                            sp = int(base + i)
                            ch = sp // GCH
                            if ch > waited_ch[st]:
                                tensor.wait_ge(
                                    (s_glo if st == 0 else s_ghi)[ch % NSEM],
                                    16 * (ch // NSEM + 1),
                                )
                                waited_ch[st] = ch
                            rhs = ring[:, (sp % RB) * D : (sp % RB) * D + D]
                            grp = g_blk // GW
                            if g_blk % GW == 0:
                                tensor.wait_ge(s_oh, grp + 1)
                            if done == 0 and t >= 2:
                                tensor.wait_ge(s_s, t - 1)
                            lhsT = oh_sb[grp % OHR][:, (g_blk % GW) * P : (g_blk % GW + 1) * P]
                            mm = tensor.matmul(
                                pagg[t % 2][:], lhsT, rhs,
                                start=(done == 0), stop=(done == nbt - 1),
                            )
                            incs = []
                            if done == nbt - 1:
                                incs.append((s_pa, 1))
                            if g_blk % GW == GW - 1 or g_blk == NBLK - 1:
                                incs.append((s_ohc, 1))
                            if sp % GCH == GCH - 1 or sp == NS - 1:
                                incs.append((s_clo if st == 0 else s_chi, GCH))
                            attach(mm, *incs)
                            done += 1
                            g_blk += 1
                    if t >= 1:
                        extras(t - 1)
                extras(T - 1)
                wapply(T - 1)
                flush()
                # stats tail: ps1 row -> columns
                tensor.wait_ge(s_t1, 1)
                for k in range(2):
                    mm = tensor.matmul(
                        pcol[:, k : k + 1],
                        t1_sb[0:1, k * P : (k + 1) * P],
                        ones1_sb[0:1, 0:1],
                        start=True, stop=True,
                    )
                    attach(mm, *([(s_pscol, 1)] if k == 1 else []))
                flush()
                # mu_agg = W @ mu_s (columns), after the AllReduce
                tensor.wait_ge(s_mu, 1)
                for j in range(2):
                    for i in range(2):
                        mm = tensor.matmul(
                            pcol[:, 2 + j : 3 + j],
                            Wt_sb[:, i * D + j * P : i * D + j * P + P],
                            mu_s_sb[:, i : i + 1],
                            start=(i == 0), stop=(i == 1),
                        )
                        attach(mm, *([(s_pmu, 1)] if (j == 1 and i == 1) else []))
                flush()
                _barrier(tensor, _it, clear=False)

        # ---------------- DVE
        @block.vector
        def _(vector):
            for _it in range(REP):
                vector.wait_ge(s_ld, 16 * NCONST)
                vector.wait_ge(s_ld4, 16)
                grp_done = 0
                tile_end_grp = []
                acc = 0
                for t in range(T):
                    acc += int(Blo[t] + Bhi[t])
                    tile_end_grp.append(-(-acc // GW))

                def emit_groups(upto):
                    nonlocal grp_done
                    while grp_done < min(upto, NGRP):
                        g0 = grp_done
                        if g0 >= OHR:
                            vector.wait_ge(s_ohc, g0 - OHR + 1)
                        nblocks = min(GW, NBLK - g0 * GW)
                        vector.tensor_tensor(
                            out=oh_sb[g0 % OHR][:, : nblocks * P],
                            in0=seg_sb[:, g0 * GW : g0 * GW + nblocks].to_broadcast(
                                [P, nblocks, P]
                            ),
                            in1=iota_sb[:, : nblocks * P],
                            op=mybir.AluOpType.is_equal,
                        ).then_inc(s_oh, 1)
                        grp_done += 1

                for t in range(T):
                    emit_groups(tile_end_grp[min(t + 1, T - 1)])
                    vector.wait_ge(s_pa, t + 1)
                    if t >= SR:
                        vector.wait_ge(s_t, t - SR + 1)
                    vector.tensor_mul(
                        s_sb[t % SR][:],
                        pagg[t % 2][:],
                        dinv_sb[:, t : t + 1].to_broadcast([P, D]),
                    ).then_inc(s_s, 1)
                    if t >= 1:
                        u = t - 1
                        vector.wait_ge(s_t, u + 1)
                        vector.tensor_copy(
                            out=sT_sb[u % SR2][:], in_=psT[u % 2][:]
                        ).then_inc(s_tc, 1)
                    if t >= 2:
                        u = t - 2
                        vector.wait_ge(s_w, u + 1)
                        vector.tensor_copy(
                            out=agg_sb[:, u * D : (u + 1) * D], in_=paggT[u % 2][:]
                        ).then_inc(s_ac, 1)
                emit_groups(NGRP)
                vector.wait_ge(s_t, T)
                vector.tensor_copy(
                    out=sT_sb[(T - 1) % SR2][:], in_=psT[(T - 1) % 2][:]
                ).then_inc(s_tc, 1)
                for u in (T - 2, T - 1):
                    vector.wait_ge(s_w, u + 1)
                    vector.tensor_copy(
                        out=agg_sb[:, u * D : (u + 1) * D], in_=paggT[u % 2][:]
                    ).then_inc(s_ac, 1)
                # stats pack
                vector.wait_ge(s_st, 1)
                vector.tensor_copy(out=t1_sb[:], in_=ps1[:]).then_inc(s_t1, 1)
                vector.wait_ge(s_pscol, 1)
                vector.tensor_copy(out=stin_sb[:, 0:2], in_=pcol[:, 0:2]).then_inc(
                    s_stsb, 1
                )
                vector.wait_ge(s_sq, 2 * T)
                vector.tensor_reduce(
                    stin_sb[:, 2:3], sqcols[:, 0:T],
                    mybir.AxisListType.X, mybir.AluOpType.add,
                ).then_inc(s_stsb, 1)
                vector.tensor_reduce(
                    stin_sb[:, 3:4], sqcols[:, T : 2 * T],
                    mybir.AxisListType.X, mybir.AluOpType.add,
                ).then_inc(s_stsb, 1)
                # post-AllReduce epilogue coefficients
                # NOTE: the coefficient chain below is tiny [P,2] ops with
                # same-engine RAW dependencies; raw-bass DVE pipelines reads
                # ahead of the previous op's write, so drain between them.
                vector.wait_ge(s_ldst, 16)
                vector.tensor_scalar_mul(mu_s_sb[:], stout_sb[:, 0:2], 1.0 / N).then_inc(
                    s_mu, 1
                )
                if DEBUG:
                    vector.drain()
                    vector.tensor_copy(out=msf_sb[:], in_=mu_s_sb[:])
                vector.wait_ge(s_pmu, 1)
                vector.tensor_copy(out=mu_a_sb[:], in_=pcol[:, 2:4])
                vector.tensor_scalar_mul(tmp1_sb[:], stout_sb[:, 2:4], 1.0 / N)
                vector.drain()
                vector.tensor_mul(tmp2_sb[:], mu_a_sb[:], mu_a_sb[:])
                vector.drain()
                vector.tensor_sub(tmp1_sb[:], tmp1_sb[:], tmp2_sb[:])
                vector.drain()
                vector.tensor_scalar_add(tmp1_sb[:], tmp1_sb[:], BN_EPS)
                vector.drain()
                vector.reciprocal(tmp1_sb[:], tmp1_sb[:])
                vector.drain().then_inc(s_var, 1)
                vector.wait_ge(s_rstd, 1)
                vector.tensor_mul(ab_sb[:, 0:2], gbc_sb[:, 0:2], rstd_sb[:])
                vector.drain()
                vector.tensor_mul(tmp2_sb[:], ab_sb[:, 0:2], mu_a_sb[:])
                vector.drain()
                vector.tensor_sub(ab_sb[:, 2:4], gbc_sb[:, 2:4], tmp2_sb[:])
                vector.drain().then_inc(s_ab, 1)
                _barrier(vector, _it, clear=False)

        # ---------------- ACT
        @block.scalar
        def _(scalar):
            for _it in range(REP):
                for t in range(T):
                    scalar.wait_ge(s_ac, t + 1)
                    for k in range(2):
                        scalar.activation(
                            sqscr[:],
                            agg_sb[:, t * D + k * P : t * D + (k + 1) * P],
                            mybir.ActivationFunctionType.Square,
                            accum_out=sqcols[:, k * T + t : k * T + t + 1],
                        ).then_inc(s_sq, 1)
                scalar.wait_ge(s_var, 1)
                scalar.activation(
                    rstd_sb[:], tmp1_sb[:], mybir.ActivationFunctionType.Sqrt
                ).then_inc(s_rstd, 1)
                scalar.wait_ge(s_ab, 1)
                # batched epilogue: one strided activation per (store group, k)
                # — A/B are per-partition and shared across tiles of a k-half.
                for g in range(NSTG):
                    t0g, t1g = g * ST, min(T, (g + 1) * ST)
                    nt = t1g - t0g
                    for k in range(2):
                        src = agg_sb[:, t0g * D : t1g * D].rearrange(
                            "p (t d) -> p t d", d=D
                        )[:, :, k * P : (k + 1) * P]
                        dst = y_big[
                            :, k * T * P + t0g * P : k * T * P + t1g * P
                        ].rearrange("p (t n) -> p t n", n=P)
                        act = scalar.activation(
                            dst,
                            src,
                            mybir.ActivationFunctionType.Relu,
                            bias=ab_sb[:, 2 + k : 3 + k],
                            scale=ab_sb[:, k : k + 1],
                        )
                        if k == 1:
                            act.then_inc(s_yr, 1)
                _barrier(scalar, _it, clear=False)

    nc.compile()
    return nc


# ------------------------------------------------------------------ driver

_CACHE = {}


def build_all(x, edge_index, W, bias, gamma, beta, M=8):
    x = np.asarray(x, np.float32)
    W = np.asarray(W, np.float32)
    gamma = np.asarray(gamma, np.float32)
    beta = np.asarray(beta, np.float32)
    in_maps, meta = _preprocess(x, edge_index, W, gamma, beta, M)
    sig = (x.shape, meta["LB"], meta["HB"], tuple(meta["Blo"]), tuple(meta["Bhi"]), DEBUG)
    if _CACHE.get("sig") != sig:
        _CACHE["nc"] = _build_program(meta)
        _CACHE["sig"] = sig
    return _CACHE["nc"], in_maps, meta


def assemble_output(per_core_results, meta):
    S, M = meta["S"], meta["M"]
    out = np.empty((M * S, 2 * P), np.float32)
    for m in range(M):
        ym = per_core_results[m]["y"]  # [2, P, S]
        out[m * S : (m + 1) * S] = ym.transpose(2, 0, 1).reshape(S, 2 * P)
    return out


def kernel(x, edge_index, W, bias, gamma, beta):
    nc, in_maps, meta = build_all(x, edge_index, W, bias, gamma, beta)
    from concourse.bass_utils import run_bass_kernel_spmd

    res = run_bass_kernel_spmd(nc, in_maps, list(range(meta["M"])))
    return assemble_output(res.results, meta)


# revision 48
# speedup vs baseline: 1.2157x; 1.2157x over previous
"""DenseGCNLayer (GCNConv + BatchNorm + ReLU) on 8 TRN2 NeuronCores.

Self-contained kernel: takes the FULL inputs, shards target nodes across 8
cores, runs a raw-bass SPMD program (bf16 compute, f32 accumulation), returns
the full [N, D] float32 output.

Strategy (v2, no h-AllGather): the linear transform commutes with the
aggregation, so compute  s[c] = dinv[c] * (sum_{r->c} dinv[r] x[r])  first
(self-loops folded into the edge list), then  agg = s @ W.T,  then BatchNorm
(global stats via a tiny [P,4] AllReduce) + ReLU.  Every core gets the SAME
replicated gather table  xs = dinv[:,None]*x  (bf16, host-prepared), so the
only on-device communication is the 2KB BN-stats AllReduce.

Per core: 6250 target nodes in 49 tiles of 128; incoming edges (+self loops)
are grouped per (tile, src-half), padded to 128-edge blocks, streamed via
1024-row dma_gather chunks (4 SWDGE queues) into two 48-block ring buffers.
Each 128-edge block is reduced into its target tile by a PE matmul with a
0/1 one-hot (edge-slot -> target-slot) built on DVE.  Per tile, PE also
transposes s (via identity matmul) and applies W with stationary W chunks;
ACT squares agg for variance stats and runs the fused relu(A*x+B) epilogue
into one big bf16 buffer that is stored in 8-tile DMA batches.  Output is
produced transposed ([2,128,S] bf16 per core) and fixed up on host.
"""
from contextlib import ExitStack

import numpy as np
import ml_dtypes

import concourse.bass as bass
import concourse.bacc as bacc
import concourse.mybir as mybir
from concourse.library_config import mlp

P = 128
GCH = 8        # blocks per gather instruction
RB = 48         # gather ring capacity per stream, in 128-edge blocks
GW = 8          # one-hot blocks built per DVE op
NSEM = 8        # rotating DMA sems per stream
OHR = 8         # one-hot ring depth (groups)
NQ = 4          # SWDGE queues
SR = 4          # s ring
SR2 = 4         # sT ring
BN_EPS = 1e-5
BF16 = ml_dtypes.bfloat16
NCONST = 7      # small const loads (Wt x2, gbc, iota, ident, ones1, dinv)
CHEAD = 8       # idx-map head chunks per stream loaded before gathers start
DEBUG = False   # add a dbg output dumping stats/coef buffers + agg tiles


# ---------------------------------------------------------------- host prep

def _preprocess(x, edge_index, W, gamma, beta, M=8):
    N, D = x.shape
    S = N // M
    assert S * M == N
    T = (S + P - 1) // P
    NH = N // 2
    src_e = np.asarray(edge_index[0], np.int64)
    tgt_e = np.asarray(edge_index[1], np.int64)
    deg = (np.bincount(tgt_e, minlength=N) + 1).astype(np.float64)
    dinv = (1.0 / np.sqrt(deg)).astype(np.float32)

    # fold self-loops into the edge stream
    loops = np.arange(N, dtype=np.int64)
    src = np.concatenate([src_e, loops])
    tgt = np.concatenate([tgt_e, loops])

    core_of = tgt // S
    loc = tgt - core_of * S
    tl = loc // P
    slot = loc % P
    ishi = (src >= NH).astype(np.int64)
    key = (core_of * T + tl) * 2 + ishi
    order = np.argsort(key, kind="stable")
    cnt = np.bincount(key, minlength=M * T * 2).reshape(M, T, 2)
    starts = np.zeros(M * T * 2 + 1, np.int64)
    np.cumsum(cnt.reshape(-1), out=starts[1:])

    nblk = -(-cnt // P)                       # ceil, [M, T, 2]
    Blo = np.maximum(nblk[:, :, 0].max(axis=0), 1)   # [T]
    Bhi = np.maximum(nblk[:, :, 1].max(axis=0), 1)
    assert Blo.max() <= RB and Bhi.max() <= RB
    LB, HB = int(Blo.sum()), int(Bhi.sum())
    lo_start = np.zeros(T, np.int64); np.cumsum(Blo[:-1], out=lo_start[1:])
    hi_start = np.zeros(T, np.int64); np.cumsum(Bhi[:-1], out=hi_start[1:])

    # consumption order: per tile, lo blocks then hi blocks
    cons = []
    for t in range(T):
        for i in range(int(Blo[t])):
            cons.append((0, int(lo_start[t] + i), t))
        for i in range(int(Bhi[t])):
            cons.append((1, int(hi_start[t] + i), t))
    NBLK = len(cons)

    # gather schedule: fixed GCH-block chunk instructions per stream (ring
    # position == stream block index, no skips), issued in consumption
    # order of the first block of each chunk.
    LBc = -(-LB // GCH)
    HBc = -(-HB // GCH)
    runs = []                      # (st, chunk_id, b0, nb, rpos)
    first_need = {}
    for step, (st, sp, _t) in enumerate(cons):
        key = (st, sp // GCH)
        if key not in first_need:
            first_need[key] = step
    for st, ch in sorted(first_need, key=first_need.get):
        nblks = (LBc, HBc)[st] * 0 + min(GCH, (LB, HB)[st] - ch * GCH)
        runs.append((st, ch, ch * GCH, int(nblks), ch * GCH))
    assert len(runs) == LBc + HBc

    shared = {}
    xs = (np.asarray(x, np.float32) * dinv[:, None]).astype(BF16)
    shared["tab"] = np.ascontiguousarray(xs.reshape(2, NH, D))
    shared["Wt"] = np.ascontiguousarray(W.T.astype(BF16)).reshape(2, P, D)
    gbc = np.zeros((P, 4), np.float32)
    gbc[:, 0], gbc[:, 1] = gamma[:P], gamma[P:]
    gbc[:, 2], gbc[:, 3] = beta[:P], beta[P:]
    shared["gbc"] = gbc
    shared["iota"] = np.ascontiguousarray(
        np.broadcast_to(np.tile(np.arange(P), GW).astype(BF16), (P, GW * P))
    )
    shared["ident"] = np.eye(P, dtype=BF16)
    shared["ones1"] = np.ones((P, 1), BF16)

    in_maps = []
    for m in range(M):
        # pad gather slots with row 0 (seg=-1 keeps them out of the one-hot)
        lo_src = np.empty(LB * P, np.int16)
        lo_slot = np.full(LB * P, -1.0, np.float32)
        hi_src = np.empty(HB * P, np.int16)
        hi_slot = np.full(HB * P, -1.0, np.float32)
        lo_src[:] = 0
        hi_src[:] = 0
        for t in range(T):
            for h, bsrc, bslot, bstart in (
                (0, lo_src, lo_slot, lo_start[t]),
                (1, hi_src, hi_slot, hi_start[t]),
            ):
                k = (m * T + t) * 2 + h
                e = order[starts[k] : starts[k + 1]]
                n = len(e)
                off = int(bstart) * P
                bsrc[off : off + n] = (src[e] - (NH if h else 0)).astype(np.int16)
                bslot[off : off + n] = slot[e].astype(np.float32)

        # NOTE: tried -1 pads on chunk-trailing suffixes (the Q7 shrinks
        # num_idxs over trailing negatives, which would skip their DMA
        # traffic) — it HANGS the device even with >=16 real indices kept
        # per chunk.  Any num_idxs shrink below the instruction's static
        # count appears to break the 16-engine sem-completion contract on
        # this runtime.  Keep 0-pads (gather row 0, harmless).

        def wrap_idx(flat):
            # idx j -> partition j%16, col j//16; replicate x8 across partitions
            a = flat.reshape(-1, 16).T                               # [16, cols]
            out = np.concatenate([a] * 8, axis=0)                    # [128, cols]
            return np.ascontiguousarray(out)

        seg = np.full((P, NBLK), -1.0, np.float32)
        for g_blk, (st, sp, _t) in enumerate(cons):
            arr = lo_slot if st == 0 else hi_slot
            seg[:, g_blk] = arr[sp * P : (sp + 1) * P]

        dinv_t = np.ones((P, T), np.float32)
        dm = dinv[m * S : (m + 1) * S]
        for t in range(T):
            rows = min(P, S - t * P)
            dinv_t[:rows, t] = dm[t * P : t * P + rows]

        in_maps.append(
            dict(
                idx_lo=wrap_idx(lo_src), idx_hi=wrap_idx(hi_src),
                seg=seg.astype(BF16), dinv_t=dinv_t, **shared,
            )
        )

    meta = dict(
        N=N, D=D, M=M, S=S, T=T, NH=NH,
        Blo=Blo, Bhi=Bhi, LB=LB, HB=HB,
        lo_start=lo_start, hi_start=hi_start, cons=cons,
        runs=runs,
    )
    return in_maps, meta


# ------------------------------------------------------------- bass program

def _build_program(meta, REP=1):
    N, D, S, T = meta["N"], meta["D"], meta["S"], meta["T"]
    NH, M = meta["NH"], meta["M"]
    cons, runs = meta["cons"], meta["runs"]
    Blo, Bhi = meta["Blo"], meta["Bhi"]
    lo_start, hi_start = meta["lo_start"], meta["hi_start"]
    LB, HB = meta["LB"], meta["HB"]
    NBLK = len(cons)
    NGRP = -(-NBLK // GW)
    bf = mybir.dt.bfloat16
    f32 = mybir.dt.float32

    nc = bacc.Bacc(num_devices=M, num_swdge_queues=NQ, detect_race_conditions=False)

    tab_d = nc.declare_dram_parameter("tab", [2, NH, D], bf, isOutput=False)
    idx_lo_d = nc.declare_dram_parameter("idx_lo", [P, LB * 8], mybir.dt.int16, isOutput=False)
    idx_hi_d = nc.declare_dram_parameter("idx_hi", [P, HB * 8], mybir.dt.int16, isOutput=False)
    seg_d = nc.declare_dram_parameter("seg", [P, NBLK], bf, isOutput=False)
    dinv_d = nc.declare_dram_parameter("dinv_t", [P, T], f32, isOutput=False)
    Wt_d = nc.declare_dram_parameter("Wt", [2, P, D], bf, isOutput=False)
    gbc_d = nc.declare_dram_parameter("gbc", [P, 4], f32, isOutput=False)
    iota_d = nc.declare_dram_parameter("iota", [P, GW * P], bf, isOutput=False)
    ident_d = nc.declare_dram_parameter("ident", [P, P], bf, isOutput=False)
    ones1_d = nc.declare_dram_parameter("ones1", [P, 1], bf, isOutput=False)
    y_d = nc.declare_dram_parameter("y", [2, P, S], bf, isOutput=True)
    if DEBUG:
        dbg_d = nc.declare_dram_parameter("dbg", [P, 20 + 3 * D], f32, isOutput=True)

    st_in = nc.dram_tensor("st_in", [P, 4], f32)
    st_out = nc.dram_tensor("st_out", [P, 4], f32, addr_space="Shared")

    with ExitStack() as ctx:
        sb = lambda name, shape, dt: ctx.enter_context(nc.sbuf_tensor(name, shape, dt))
        ring_lo = sb("rlo", [P, RB * D], bf)
        ring_hi = sb("rhi", [P, RB * D], bf)
        oh_sb = [sb(f"oh{r}", [P, GW * P], bf) for r in range(OHR)]
        s_sb = [sb(f"s{r}", [P, D], bf) for r in range(SR)]
        sT_sb = [sb(f"sT{r}", [P, D], bf) for r in range(SR2)]
        agg_sb = sb("agg_sb", [P, T * D], f32)
        sqcols = sb("sqcols", [P, 2 * T], f32)
        sqscr = sb("sqscr", [P, P], f32)
        idx_lo_sb = sb("idx_lo_sb", [P, LB * 8], mybir.dt.int16)
        idx_hi_sb = sb("idx_hi_sb", [P, HB * 8], mybir.dt.int16)
        seg_sb = sb("seg_sb", [P, NBLK], bf)
        dinv_sb = sb("dinv_sb", [P, T], f32)
        Wt_sb = sb("Wt_sb", [P, 2 * D], bf)
        gbc_sb = sb("gbc_sb", [P, 4], f32)
        iota_sb = sb("iota_sb", [P, GW * P], bf)
        ident_sb = sb("ident_sb", [P, P], bf)
        ones1_sb = sb("ones1_sb", [P, 1], bf)
        t1_sb = sb("t1_sb", [1, D], bf)
        mu_s_sb = sb("mu_s_sb", [P, 2], bf)
        stin_sb = sb("stin_sb", [P, 4], f32)
        stout_sb = sb("stout_sb", [P, 4], f32)
        tmp1_sb = sb("tmp1_sb", [P, 2], f32)
        tmp2_sb = sb("tmp2_sb", [P, 2], f32)
        mu_a_sb = sb("mu_a_sb", [P, 2], f32)
        msf_sb = sb("msf_sb", [P, 2], f32)
        rstd_sb = sb("rstd_sb", [P, 2], f32)
        ab_sb = sb("ab_sb", [P, 4], f32)
        y_big = sb("y_big", [P, 2 * T * P], bf)  # col = k*T*P + t*P + n

        pagg = [ctx.enter_context(nc.psum_tensor(f"pagg{r}", [P, D], f32)) for r in range(2)]
        psT = [ctx.enter_context(nc.psum_tensor(f"psT{r}", [P, D], bf)) for r in range(2)]
        paggT = [ctx.enter_context(nc.psum_tensor(f"paggT{r}", [P, D], f32)) for r in range(2)]
        ps1 = ctx.enter_context(nc.psum_tensor("ps1", [1, D], f32))
        pcol = ctx.enter_context(nc.psum_tensor("pcol", [P, 4], f32))

        sem = lambda name: ctx.enter_context(nc.semaphore(name))
        s_ld = sem("s_ld")
        s_ld2 = sem("s_ld2")
        s_ld3 = sem("s_ld3")
        s_ld4 = sem("s_ld4")
        s_glo = [sem(f"s_glo{r}") for r in range(NSEM)]
        s_ghi = [sem(f"s_ghi{r}") for r in range(NSEM)]
        s_clo = sem("s_clo")
        s_chi = sem("s_chi")
        s_oh = sem("s_oh")
        s_ohc = sem("s_ohc")
        s_pa = sem("s_pa")      # PE: pagg[t] accumulated
        s_s = sem("s_s")        # DVE: s_sb[t] written (also frees pagg[t])
        s_t = sem("s_t")        # PE: psT[t] written (also frees s_sb[t])
        s_tc = sem("s_tc")      # DVE: sT_sb[t] written (also frees psT[t])
        s_w = sem("s_w")        # PE: paggT[t] written
        s_ac = sem("s_ac")      # DVE: agg_sb tile t written (frees paggT[t])
        s_sq = sem("s_sq")      # ACT: square chunk done
        s_st = sem("s_st")      # PE: ps1 stats accumulation stopped
        s_t1 = sem("s_t1")      # DVE: t1_sb (=ps1) copied
        s_pscol = sem("s_pscol")
        s_stsb = sem("s_stsb")
        s_stst = sem("s_stst")
        s_ldst = sem("s_ldst")
        s_mu = sem("s_mu")
        s_pmu = sem("s_pmu")
        s_var = sem("s_var")
        s_rstd = sem("s_rstd")
        s_ab = sem("s_ab")
        s_yr = sem("s_yr")
        s_ye = sem("s_ye")
        cc = sem("cc")
        b1 = ctx.enter_context(nc.semaphore("b1"))
        b2 = ctx.enter_context(nc.semaphore("b2"))
        work_sems = (
            [s_ld, s_ld2, s_ld3, s_ld4, s_clo, s_chi, s_oh, s_ohc, s_pa, s_s, s_t, s_tc,
             s_w, s_ac, s_sq, s_st, s_t1, s_pscol, s_stsb, s_stst, s_ldst,
             s_mu, s_pmu, s_var, s_rstd, s_ab, s_yr, s_ye, cc]
            + s_glo + s_ghi
        )

        def _barrier(eng, it, clear=False):
            if REP == 1:
                return
            eng.sem_inc(b1, 1)
            eng.wait_ge(b1, 5 * (it + 1))
            if clear:
                for ws in work_sems:
                    eng.sem_clear(ws)
                eng.sem_inc(b2, 1)
            eng.wait_ge(b2, it + 1)

        block = ctx.enter_context(nc.Block())

        ST = 8  # tiles per output store DMA
        NSTG = -(-T // ST)

        # ---------------- SP: all HWDGE loads/stores
        @block.sync
        def _(sync):
            for _it in range(REP):
                hc = CHEAD * GCH * 8   # idx head columns per stream
                for dram, sbuf in ((idx_lo_d, idx_lo_sb), (idx_hi_d, idx_hi_sb)):
                    w = min(hc, dram.shape[1])
                    sync.dma_start(out=sbuf[:, 0:w], in_=dram[:, 0:w]).then_inc(
                        s_ld3, 16
                    )
                sync.dma_start(out=seg_sb[:], in_=seg_d[:]).then_inc(s_ld4, 16)
                for k in range(2):
                    sync.dma_start(
                        out=Wt_sb[:, k * D : (k + 1) * D], in_=Wt_d[k]
                    ).then_inc(s_ld, 16)
                for dram, sbuf in (
                    (gbc_d, gbc_sb),
                    (iota_d, iota_sb),
                    (ident_d, ident_sb),
                    (ones1_d, ones1_sb),
                    (dinv_d, dinv_sb),
                ):
                    sync.dma_start(out=sbuf[:], in_=dram[:]).then_inc(s_ld, 16)
                for dram, sbuf in ((idx_lo_d, idx_lo_sb), (idx_hi_d, idx_hi_sb)):
                    if dram.shape[1] > hc:
                        sync.dma_start(
                            out=sbuf[:, hc:], in_=dram[:, hc:]
                        ).then_inc(s_ld2, 16)
                    else:
                        sync.sem_inc(s_ld2, 16)
                sync.wait_ge(s_stsb, 3)
                sync.dma_start(out=st_in[:], in_=stin_sb[:]).then_inc(s_stst, 16)
                sync.wait_ge(cc, 1)
                sync.dma_start(out=stout_sb[:], in_=st_out[:]).then_inc(s_ldst, 16)
                for g in range(NSTG):
                    t1g = min(T, (g + 1) * ST)
                    c0, c1 = g * ST * P, min(S, t1g * P)
                    sync.wait_ge(s_yr, g + 1)
                    sync.dma_start(
                        out=y_d[:, :, c0:c1].rearrange("k p m -> p k m"),
                        in_=y_big[:].rearrange("p (k m) -> p k m", k=2)[:, :, c0:c1],
                    ).then_inc(s_ye, 16)
                sync.wait_ge(s_ye, 16 * NSTG)
                if DEBUG:
                    sync.wait_ge(s_ab, 1)
                    for off, sbuf in (
                        (0, stin_sb), (4, stout_sb), (8, ab_sb), (12, rstd_sb),
                        (14, tmp1_sb), (16, mu_a_sb), (18, msf_sb),
                    ):
                        w = sbuf.shape[1]
                        sync.dma_start(
                            out=dbg_d[:, off : off + w], in_=sbuf[:]
                        ).then_inc(s_ye, 16)
                    sync.dma_start(
                        out=dbg_d[:, 20 : 20 + D], in_=agg_sb[:, 0:D]
                    ).then_inc(s_ye, 16)
                    sync.dma_start(
                        out=dbg_d[:, 20 + D : 20 + 2 * D],
                        in_=agg_sb[:, (T - 1) * D : T * D],
                    ).then_inc(s_ye, 16)
                    sync.dma_start(
                        out=dbg_d[:, 20 + 2 * D : 20 + 3 * D],
                        in_=agg_sb[:, D : 2 * D],
                    ).then_inc(s_ye, 16)
                    sync.wait_ge(s_ye, 16 * (NSTG + 10))
                _barrier(sync, _it, clear=False)

        # ---------------- Pool: gathers + stats collective
        @block.gpsimd
        def _(gpsimd):
            for _it in range(REP):
                gpsimd.load_library(mlp)
                gpsimd.wait_ge(s_ld3, 32)
                tail_waited = False
                for r, (st, ch, b0, nb, rpos) in enumerate(runs):
                    if ch >= CHEAD and not tail_waited:
                        gpsimd.wait_ge(s_ld2, 32)
                        tail_waited = True
                    if b0 + nb > RB:
                        gpsimd.wait_ge(s_clo if st == 0 else s_chi, b0 + nb - RB)
                    idx_sb = idx_lo_sb if st == 0 else idx_hi_sb
                    ring = ring_lo if st == 0 else ring_hi
                    col0 = (b0 % RB) * D
                    gpsimd.dma_gather(
                        ring[:, col0 : col0 + nb * D].rearrange("p (k d) -> p k d", d=D),
                        tab_d[st],
                        idx_sb[:, b0 * 8 : (b0 + nb) * 8],
                        nb * P,
                        nb * P,
                        D,
                        queue_num=r % NQ,
                    ).then_inc((s_glo if st == 0 else s_ghi)[ch % NSEM], 16)
                gpsimd.wait_ge(s_stst, 16)
                gpsimd.collective_compute(
                    "AllReduce",
                    mybir.AluOpType.add,
                    replica_groups=[list(range(M))],
                    ins=[st_in[:]],
                    outs=[st_out[:]],
                ).then_inc(cc, 1)
                _barrier(gpsimd, _it, clear=True)

        # ---------------- PE
        @block.tensor
        def _(tensor):
            for _it in range(REP):
                tensor.wait_ge(s_ld, 16 * NCONST)
                g_blk = 0
                waited_ch = [-1, -1]
                pending = []

                def attach(mm, *incs):
                    # matmul sync-update slots are limited to 1; overflow rides
                    # the next matmul (consumers only ever see a later inc).
                    todo = pending + list(incs)
                    pending.clear()
                    for semh, v in todo[:1]:
                        mm.then_inc(semh, v)
                    pending.extend(todo[1:])

                def flush():
                    for semh, v in pending:
                        tensor.drain().then_inc(semh, v)
                    pending.clear()

                def wapply(u):
                    tensor.wait_ge(s_tc, u + 1)
                    if u >= 2:
                        tensor.wait_ge(s_ac, u - 1)
                    for j in range(2):
                        for i in range(2):
                            mm = tensor.matmul(
                                paggT[u % 2][:, j * P : (j + 1) * P],
                                Wt_sb[:, i * D + j * P : i * D + j * P + P],
                                sT_sb[u % SR2][:, i * P : (i + 1) * P],
                                start=(i == 0), stop=(i == 1),
                            )
                            attach(mm, *([(s_w, 1)] if (j == 1 and i == 1) else []))

                def extras(u):
                    if u >= 1:
                        wapply(u - 1)
                    tensor.wait_ge(s_s, u + 1)
                    mm = tensor.matmul(
                        ps1[:], ones1_sb[:], s_sb[u % SR][:, 0:D],
                        start=(u == 0), stop=(u == T - 1),
                    )
                    attach(mm, *([(s_st, 1)] if u == T - 1 else []))
                    if u >= 2:
                        tensor.wait_ge(s_tc, u - 1)
                    for k in range(2):
                        mm = tensor.transpose(
                            psT[u % 2][:, k * P : (k + 1) * P],
                            s_sb[u % SR][:, k * P : (k + 1) * P],
                            ident_sb[:],
                        )
                        attach(mm, *([(s_t, 1)] if k == 1 else []))

                for t in range(T):
                    nbt = int(Blo[t] + Bhi[t])
                    done = 0
                    for st, base, num in ((0, lo_start[t], Blo[t]), (1, hi_start[t], Bhi[t])):
                        ring = ring_lo if st == 0 else ring_hi
                        NS = LB if st == 0 else HB
                        for i in range(int(num)):
                            sp = int(base + i)
                            ch = sp // GCH
                            if ch > waited_ch[st]:
                                tensor.wait_ge(
                                    (s_glo if st == 0 else s_ghi)[ch % NSEM],
                                    16 * (ch // NSEM + 1),
                                )
                                waited_ch[st] = ch
                            rhs = ring[:, (sp % RB) * D : (sp % RB) * D + D]
                            grp = g_blk // GW
                            if g_blk % GW == 0:
                                tensor.wait_ge(s_oh, grp + 1)
                            if done == 0 and t >= 2:
                                tensor.wait_ge(s_s, t - 1)
                            lhsT = oh_sb[grp % OHR][:, (g_blk % GW) * P : (g_blk % GW + 1) * P]
                            mm = tensor.matmul(
                                pagg[t % 2][:], lhsT, rhs,
                                start=(done == 0), stop=(done == nbt - 1),
                            )
                            incs = []
                            if done == nbt - 1:
                                incs.append((s_pa, 1))
                            if g_blk % GW == GW - 1 or g_blk == NBLK - 1:
                                incs.append((s_ohc, 1))
                            if sp % GCH == GCH - 1 or sp == NS - 1:
                                incs.append((s_clo if st == 0 else s_chi, GCH))
                            attach(mm, *incs)
                            done += 1
                            g_blk += 1
                    if t >= 1:
                        extras(t - 1)
                extras(T - 1)
                wapply(T - 1)
                flush()
                # stats tail: ps1 row -> columns
                tensor.wait_ge(s_t1, 1)
                for k in range(2):
                    mm = tensor.matmul(
                        pcol[:, k : k + 1],
                        t1_sb[0:1, k * P : (k + 1) * P],
                        ones1_sb[0:1, 0:1],
                        start=True, stop=True,
                    )
                    attach(mm, *([(s_pscol, 1)] if k == 1 else []))
                flush()
                # mu_agg = W @ mu_s (columns), after the AllReduce
                tensor.wait_ge(s_mu, 1)
                for j in range(2):
                    for i in range(2):
                        mm = tensor.matmul(
                            pcol[:, 2 + j : 3 + j],
                            Wt_sb[:, i * D + j * P : i * D + j * P + P],
                            mu_s_sb[:, i : i + 1],
                            start=(i == 0), stop=(i == 1),
                        )
                        attach(mm, *([(s_pmu, 1)] if (j == 1 and i == 1) else []))
                flush()
                _barrier(tensor, _it, clear=False)

        # ---------------- DVE
        @block.vector
        def _(vector):
            for _it in range(REP):
                vector.wait_ge(s_ld, 16 * NCONST)
                vector.wait_ge(s_ld4, 16)
                grp_done = 0
                tile_end_grp = []
                acc = 0
                for t in range(T):
                    acc += int(Blo[t] + Bhi[t])
                    tile_end_grp.append(-(-acc // GW))

                def emit_groups(upto):
                    nonlocal grp_done
                    while grp_done < min(upto, NGRP):
                        g0 = grp_done
                        if g0 >= OHR:
                            vector.wait_ge(s_ohc, g0 - OHR + 1)
                        nblocks = min(GW, NBLK - g0 * GW)
                        vector.tensor_tensor(
                            out=oh_sb[g0 % OHR][:, : nblocks * P],
                            in0=seg_sb[:, g0 * GW : g0 * GW + nblocks].to_broadcast(
                                [P, nblocks, P]
                            ),
                            in1=iota_sb[:, : nblocks * P],
                            op=mybir.AluOpType.is_equal,
                        ).then_inc(s_oh, 1)
                        grp_done += 1

                for t in range(T):
                    emit_groups(tile_end_grp[min(t + 1, T - 1)])
                    vector.wait_ge(s_pa, t + 1)
                    if t >= SR:
                        vector.wait_ge(s_t, t - SR + 1)
                    vector.tensor_mul(
                        s_sb[t % SR][:],
                        pagg[t % 2][:],
                        dinv_sb[:, t : t + 1].to_broadcast([P, D]),
                    ).then_inc(s_s, 1)
                    if t >= 1:
                        u = t - 1
                        vector.wait_ge(s_t, u + 1)
                        vector.tensor_copy(
                            out=sT_sb[u % SR2][:], in_=psT[u % 2][:]
                        ).then_inc(s_tc, 1)
                    if t >= 2:
                        u = t - 2
                        vector.wait_ge(s_w, u + 1)
                        vector.tensor_copy(
                            out=agg_sb[:, u * D : (u + 1) * D], in_=paggT[u % 2][:]
                        ).then_inc(s_ac, 1)
                emit_groups(NGRP)
                vector.wait_ge(s_t, T)
                vector.tensor_copy(
                    out=sT_sb[(T - 1) % SR2][:], in_=psT[(T - 1) % 2][:]
                ).then_inc(s_tc, 1)
                for u in (T - 2, T - 1):
                    vector.wait_ge(s_w, u + 1)
                    vector.tensor_copy(
                        out=agg_sb[:, u * D : (u + 1) * D], in_=paggT[u % 2][:]
                    ).then_inc(s_ac, 1)
                # stats pack
                vector.wait_ge(s_st, 1)
                vector.tensor_copy(out=t1_sb[:], in_=ps1[:]).then_inc(s_t1, 1)
                vector.wait_ge(s_pscol, 1)
                vector.tensor_copy(out=stin_sb[:, 0:2], in_=pcol[:, 0:2]).then_inc(
                    s_stsb, 1
                )
                vector.wait_ge(s_sq, 2 * T)
                vector.tensor_reduce(
                    stin_sb[:, 2:3], sqcols[:, 0:T],
                    mybir.AxisListType.X, mybir.AluOpType.add,
                ).then_inc(s_stsb, 1)
                vector.tensor_reduce(
                    stin_sb[:, 3:4], sqcols[:, T : 2 * T],
                    mybir.AxisListType.X, mybir.AluOpType.add,
                ).then_inc(s_stsb, 1)
                # post-AllReduce epilogue coefficients
                # NOTE: the coefficient chain below is tiny [P,2] ops with
                # same-engine RAW dependencies; raw-bass DVE pipelines reads
                # ahead of the previous op's write, so drain between them.
                vector.wait_ge(s_ldst, 16)
                vector.tensor_scalar_mul(mu_s_sb[:], stout_sb[:, 0:2], 1.0 / N).then_inc(
                    s_mu, 1
                )
                if DEBUG:
                    vector.drain()
                    vector.tensor_copy(out=msf_sb[:], in_=mu_s_sb[:])
                vector.wait_ge(s_pmu, 1)
                vector.tensor_copy(out=mu_a_sb[:], in_=pcol[:, 2:4])
                vector.tensor_scalar_mul(tmp1_sb[:], stout_sb[:, 2:4], 1.0 / N)
                vector.drain()
                vector.tensor_mul(tmp2_sb[:], mu_a_sb[:], mu_a_sb[:])
                vector.drain()
                vector.tensor_sub(tmp1_sb[:], tmp1_sb[:], tmp2_sb[:])
                vector.drain()
                vector.tensor_scalar_add(tmp1_sb[:], tmp1_sb[:], BN_EPS)
                vector.drain()
                vector.reciprocal(tmp1_sb[:], tmp1_sb[:])
                vector.drain().then_inc(s_var, 1)
                vector.wait_ge(s_rstd, 1)
                vector.tensor_mul(ab_sb[:, 0:2], gbc_sb[:, 0:2], rstd_sb[:])
                vector.drain()
                vector.tensor_mul(tmp2_sb[:], ab_sb[:, 0:2], mu_a_sb[:])
                vector.drain()
                vector.tensor_sub(ab_sb[:, 2:4], gbc_sb[:, 2:4], tmp2_sb[:])
                vector.drain().then_inc(s_ab, 1)
                _barrier(vector, _it, clear=False)

        # ---------------- ACT
        @block.scalar
        def _(scalar):
            for _it in range(REP):
                for t in range(T):
                    scalar.wait_ge(s_ac, t + 1)
                    for k in range(2):
                        scalar.activation(
                            sqscr[:],
                            agg_sb[:, t * D + k * P : t * D + (k + 1) * P],
                            mybir.ActivationFunctionType.Square,
                            accum_out=sqcols[:, k * T + t : k * T + t + 1],
                        ).then_inc(s_sq, 1)
                scalar.wait_ge(s_var, 1)
                scalar.activation(
                    rstd_sb[:], tmp1_sb[:], mybir.ActivationFunctionType.Sqrt
                ).then_inc(s_rstd, 1)
                scalar.wait_ge(s_ab, 1)
                # batched epilogue: one strided activation per (store group, k)
                # — A/B are per-partition and shared across tiles of a k-half.
                for g in range(NSTG):
                    t0g, t1g = g * ST, min(T, (g + 1) * ST)
                    nt = t1g - t0g
                    for k in range(2):
                        src = agg_sb[:, t0g * D : t1g * D].rearrange(
                            "p (t d) -> p t d", d=D
                        )[:, :, k * P : (k + 1) * P]
                        dst = y_big[
                            :, k * T * P + t0g * P : k * T * P + t1g * P
                        ].rearrange("p (t n) -> p t n", n=P)
                        act = scalar.activation(
                            dst,
                            src,
                            mybir.ActivationFunctionType.Relu,
                            bias=ab_sb[:, 2 + k : 3 + k],
                            scale=ab_sb[:, k : k + 1],
                        )
                        if k == 1:
                            act.then_inc(s_yr, 1)
                _barrier(scalar, _it, clear=False)

    nc.compile()
    return nc


# ------------------------------------------------------------------ driver

_CACHE = {}


def build_all(x, edge_index, W, bias, gamma, beta, M=8):
    x = np.asarray(x, np.float32)
    W = np.asarray(W, np.float32)
    gamma = np.asarray(gamma, np.float32)
    beta = np.asarray(beta, np.float32)
    in_maps, meta = _preprocess(x, edge_index, W, gamma, beta, M)
    sig = (x.shape, meta["LB"], meta["HB"], tuple(meta["Blo"]), tuple(meta["Bhi"]), DEBUG)
    if _CACHE.get("sig") != sig:
        _CACHE["nc"] = _build_program(meta)
        _CACHE["sig"] = sig
    return _CACHE["nc"], in_maps, meta


def assemble_output(per_core_results, meta):
    S, M = meta["S"], meta["M"]
    out = np.empty((M * S, 2 * P), np.float32)
    for m in range(M):
        ym = per_core_results[m]["y"]  # [2, P, S]
        out[m * S : (m + 1) * S] = ym.transpose(2, 0, 1).reshape(S, 2 * P)
    return out


def kernel(x, edge_index, W, bias, gamma, beta):
    nc, in_maps, meta = build_all(x, edge_index, W, bias, gamma, beta)
    from concourse.bass_utils import run_bass_kernel_spmd

    res = run_bass_kernel_spmd(nc, in_maps, list(range(meta["M"])))
    return assemble_output(res.results, meta)
